# revision 21
# baseline (speedup 1.0000x reference)
"""Trainium2 Bass kernel for nn_CI3addFrom01 (segment_reduce).

Reference computation:
    out[b] = sum_m softmax(preweight)[m] * max_k min_j x[b, idx[m,k,j]]
with M = 40704 antichains over DIM = 32.

Device formulation (M-axis sharded 8 ways; per core, per 128-row batch
tile, everything is one-hot matmuls + rectangular broadcast min/max +
weighted accumulation against a host-precomputed exp(preweight) table):
    G_j = xT.T @ OH_j (j=0,1,2) over 838 "set" columns
    SM = min3(G)[:714]; MX = max3(G)[32:714]; X = min3(G)[714:838]
    xcall regions, column-aligned with a host-packed weight row E:
      R1 [0:714]     = SM              1-group antichains
      R2 [714:1396]  = MX              ((i,),(j,)) / ((i,),(j,),(k,))
      R3a [1396:3380] max(S_a, P_p)    [32 x 62] singleton-vs-pair grid
      R3b [3380:5784] max(X_u, X_v)    2-level triangle-split pair-vs-pair
    weighted sum: acc[b] += sum_c E_c * xcall[b, c], E = exp(pw_packed)
    (host computes exp and the softmax denominator; invalid grid slots
    get E = 0, double-counted triangle slots get E/2)
Work is balanced across DVE (bf16 2x chain + grids + mult), ACT (fused
PSUM->SBUF copy + accumulate-reduce), and GPSIMD/Pool (grids + fused stt).
Host sums the per-core partials and divides by Z = sum(exp(pw)).
"""

import itertools
import math

import numpy as np

DIM = 32
B = 512
NCORES = 8
NPAIR_C = 62
NTRIP_C = 620
NM_C = 4
# per-m X slice padded to 32 slots (31 real pair-mins + 1 dummy whose
# packed weight is 0) so every R3b grid operand slice starts at an even
# bf16 column (4B-aligned — required for the DVE 2x_1p perf mode)
XPAD = 32
NTAB = 32 + NPAIR_C + NTRIP_C + NM_C * XPAD  # 842
NEG = -1e30

PAIRS = list(itertools.combinations(range(DIM), 2))
TRIPS = list(itertools.combinations(range(DIM), 3))
TRIPIDX = {t: i for i, t in enumerate(TRIPS)}

# R3b 2-level triangle split grids: (name, udim, vdim, uoff, voff, halved)
# offsets index into the per-m 32-slot padded X slice; all offsets/dims
# even.  A = slots [0:16], B = [16:32] (slot 31 = dummy)
R3B_GRIDS = [
    ("rAB", 16, 16, 0, 16, False),
    ("rA", 8, 8, 0, 8, False),
    ("tA1", 8, 8, 0, 0, True),
    ("tA2", 8, 8, 8, 8, True),
    ("rB", 8, 8, 16, 24, False),
    ("tB1", 8, 8, 16, 16, True),
    ("tB2", 8, 8, 24, 24, True),
]

SEG = {}
_off = 0
for _name, _ln in (
    [("R1a", 32), ("R1b", NPAIR_C), ("R1c", NTRIP_C),
     ("R2b", NPAIR_C), ("R2c", NTRIP_C),
     ("R3a", 32 * NPAIR_C)]
    + [(n, NM_C * u * v) for n, u, v, _, _, _ in R3B_GRIDS]
):
    SEG[_name] = (_off, _off + _ln)
    _off += _ln
LPACK = _off  # 5784
N_R1 = 32 + NPAIR_C + NTRIP_C  # 714
N_R2 = NPAIR_C + NTRIP_C       # 682

# --- engine split tuning knobs ---
# RMAT: R3a rows [0:RMAT] get their singleton operand materialized by ACT
# (rep-each-62 copy from PSUM) so the DVE grid max runs in bf16 2x mode;
# rows [RMAT:32] use the stride-0 broadcast operand at 1x.
RMAT = 32
# materialize the R3b grids' rep-each operand on ACT too — only pays when
# ACT has slack (i.e. with WSUM_STT on)
R3B_MAT = False
# weighted sum via DVE scalar_tensor_tensor (real HW gives it 2x_1p, so it
# fuses the multiply and the reduce at the TT-mult price and frees ACT);
# False = DVE TT-mult + ACT activation-accumulate
WSUM_STT = True
# wsum chunk boundary: [0:WSPLIT] accumulated right after R3a,
# [WSPLIT:LPACK] after the R3b grids (both DVE TT-mult + ACT accum-copy).
WSPLIT = 32 + NPAIR_C + NTRIP_C + NPAIR_C + NTRIP_C + 32 * NPAIR_C  # 3380


def _others(m):
    return [x for x in range(DIM) if x != m]


def _table_sets(core):
    sets = [(i, i, i) for i in range(DIM)]
    for p in range(NPAIR_C * core, NPAIR_C * (core + 1)):
        i, j = PAIRS[p]
        sets.append((i, j, j))
    for q in range(NTRIP_C * core, NTRIP_C * (core + 1)):
        sets.append(TRIPS[q])
    for m in range(NM_C * core, NM_C * (core + 1)):
        for x in _others(m):
            sets.append((min(m, x), max(m, x), max(m, x)))
        sets.append((0, 0, 0))  # pad slot 31: any finite value, weight 0
    return sets


_HOST_CACHE = {}


def _onehots(core):
    if ("oh", core) in _HOST_CACHE:
        return _HOST_CACHE[("oh", core)]
    sets = np.asarray(_table_sets(core), dtype=np.int64)
    oh = np.zeros((3, DIM, NTAB), dtype=np.float32)
    for j in range(3):
        oh[j, sets[:, j], np.arange(NTAB)] = 1.0
    _HOST_CACHE[("oh", core)] = oh
    return oh


def _widx_r3a(core):
    if ("r3a", core) in _HOST_CACHE:
        return _HOST_CACHE[("r3a", core)]
    g = np.full((32, NPAIR_C), -1, dtype=np.int64)
    for a in range(32):
        for pl, p in enumerate(range(NPAIR_C * core, NPAIR_C * (core + 1))):
            b, c = PAIRS[p]
            if a == b or a == c:
                continue
            tri = tuple(sorted((a, b, c)))
            g[a, pl] = 1024 + 8 * TRIPIDX[tri] + 2 + tri.index(a)
    _HOST_CACHE[("r3a", core)] = g
    return g


def _r3b_widx_id(m, ou, ov):
    tri = tuple(sorted((m, ou, ov)))
    return 1024 + 8 * TRIPIDX[tri] + (7, 5, 6)[tri.index(m)]


def _widx_r3b(core):
    """Per 2-level grid: index table [NM_C, u, v] (-1 = invalid slot).
    Slot indices address the padded 32-slot others list; slot 31 (pad)
    and diagonal cells are invalid (weight 0)."""
    if ("r3b", core) in _HOST_CACHE:
        return _HOST_CACHE[("r3b", core)]
    grids = {}
    for name, ud, vd, uo, vo, halved in R3B_GRIDS:
        g = np.full((NM_C, ud, vd), -1, dtype=np.int64)
        for ml, m in enumerate(range(NM_C * core, NM_C * (core + 1))):
            ot = _others(m)
            for u in range(ud):
                for v in range(vd):
                    iu, iv = uo + u, vo + v
                    if iu >= 31 or iv >= 31:
                        continue
                    gu, gv = ot[iu], ot[iv]
                    if gu != gv:
                        g[ml, u, v] = _r3b_widx_id(m, gu, gv)
        grids[name] = (g, halved)
    _HOST_CACHE[("r3b", core)] = grids
    return grids


def _packed_pwe(core, pw):
    """exp of the packed preweight row: the weighted-sum table E."""
    pw = np.asarray(pw, dtype=np.float64).reshape(-1)
    out = np.full(LPACK, NEG, dtype=np.float64)
    if core == 0:
        out[SEG["R1a"][0]:SEG["R1a"][1]] = pw[0:32]
    p0, p1 = 32 + 2 * NPAIR_C * core, 32 + 2 * NPAIR_C * (core + 1)
    out[SEG["R1b"][0]:SEG["R1b"][1]] = pw[p0:p1:2]
    out[SEG["R2b"][0]:SEG["R2b"][1]] = pw[p0 + 1:p1 + 1:2]
    t0, t1 = 1024 + 8 * NTRIP_C * core, 1024 + 8 * NTRIP_C * (core + 1)
    out[SEG["R1c"][0]:SEG["R1c"][1]] = pw[t0:t1:8]
    out[SEG["R2c"][0]:SEG["R2c"][1]] = pw[t0 + 1:t1 + 1:8]
    items = [("R3a", _widx_r3a(core), False)]
    r3b = _widx_r3b(core)
    for name, _, _, _, _, _ in R3B_GRIDS:
        g, halved = r3b[name]
        items.append((name, g, halved))
    for name, grid, halved in items:
        flat = grid.reshape(-1)
        vals = np.full(flat.shape, NEG, dtype=np.float64)
        ok = flat >= 0
        vals[ok] = pw[flat[ok]]
        if halved:
            vals[ok] -= math.log(2.0)
        s, e = SEG[name]
        out[s:e] = vals
    return np.exp(out)


def _expected_idx():
    acs = [((i,),) for i in range(DIM)]
    for i, j in PAIRS:
        acs.append(((i, j),))
        acs.append(((i,), (j,)))
    for i, j, k in TRIPS:
        acs += [((i, j, k),), ((i,), (j,), (k,)), ((i,), (j, k)), ((j,), (i, k)),
                ((k,), (i, j)), ((i, j), (j, k)), ((i, k), (j, k)), ((i, j), (i, k))]
    idx = np.zeros((len(acs), 3, 3), dtype=np.int32)
    for m, ac in enumerate(acs):
        groups = [list(g) + [g[-1]] * (3 - len(g)) for g in ac]
        while len(groups) < 3:
            groups.append(groups[-1])
        idx[m] = np.array(groups, dtype=np.int32)
    return idx


_NC_CACHE = {}

_WP_BUFS = 3
_JP_BUFS = 2


def _build_nc(reps=1):
    import concourse.mybir as mybir
    from concourse import bacc
    from concourse.tile import TileContext

    f32 = mybir.dt.float32
    bf16 = mybir.dt.bfloat16

    nc = bacc.Bacc(None, target_bir_lowering=False, debug=False)
    xT_d = nc.dram_tensor("xT", [DIM, B], bf16, kind="ExternalInput")
    oh_d = nc.dram_tensor("oh", [DIM, 3 * NTAB], bf16, kind="ExternalInput")
    pwe_d = nc.dram_tensor("pwe", [1, LPACK], bf16, kind="ExternalInput")
    outd_d = nc.dram_tensor("outd", [B, 1], f32, kind="ExternalOutput")
    outa_d = nc.dram_tensor("outa", [B, 1], f32, kind="ExternalOutput")
    outp_d = nc.dram_tensor("outp", [B, 1], f32, kind="ExternalOutput")

    with TileContext(nc) as tc:
        with (
            tc.tile_pool(name="const", bufs=1) as cp,
            tc.tile_pool(name="ep", bufs=2) as ep,
            tc.tile_pool(name="work", bufs=_WP_BUFS) as wp,
            tc.tile_pool(name="junkp", bufs=_JP_BUFS) as jp,
            tc.tile_pool(name="pg", bufs=1, space="PSUM") as pg_pool,
        ):
            oh_t = cp.tile([DIM, 3 * NTAB], bf16)
            xt_t = cp.tile([DIM, B], bf16)
            outd_b = cp.tile([128, 4], f32)
            outa_b = cp.tile([128, 4], f32)
            outp_b = cp.tile([128, 4], f32)
            nc.sync.dma_start(oh_t[:], oh_d[:])
            nc.sync.dma_start(xt_t[:], xT_d[:])

            rep_blocks(nc, tc, mybir, f32, bf16, cp, ep, wp, jp, pg_pool,
                       oh_t, xt_t, pwe_d, outd_b, outa_b, outp_b, reps)

            for t in range(4):
                nc.sync.dma_start(outd_d[t * 128:(t + 1) * 128, :],
                                  outd_b[:, t:t + 1])
                nc.sync.dma_start(outa_d[t * 128:(t + 1) * 128, :],
                                  outa_b[:, t:t + 1])
                nc.sync.dma_start(outp_d[t * 128:(t + 1) * 128, :],
                                  outp_b[:, t:t + 1])
    nc.finalize()
    return nc


def rep_blocks(nc, tc, mybir, f32, bf16, cp, ep, wp, jp, pg_pool,
               oh_t, xt_t, pwe_d, outd_b, outa_b, outp_b, reps):
    Alu = mybir.AluOpType
    Act = mybir.ActivationFunctionType
    for _rep in range(reps):
        # E = exp(pw_packed) broadcast to 128 partitions via DMA on the SP
        # queue (keep the ACT queue free); double-buffered across reps
        E = ep.tile([128, LPACK], bf16, tag="E")
        nc.sync.dma_start(E[:], pwe_d[0:1, :].broadcast_to([128, LPACK]))

        for t in range(4):
            # G: one PSUM tile, j-blocks at col j*1024 so every 512-col
            # matmul chunk is PSUM-bank aligned
            G = pg_pool.tile([128, 3 * 1024], f32, tag="G")
            for j in range(3):
                for s in range(0, NTAB, 512):
                    e = min(s + 512, NTAB)
                    nc.tensor.matmul(
                        G[:, j * 1024 + s: j * 1024 + e],
                        xt_t[:, t * 128:(t + 1) * 128],
                        oh_t[:, j * NTAB + s: j * NTAB + e],
                        start=True, stop=True)

            # single fused PSUM->SBUF bf16 copy of all three j-blocks
            c012 = wp.tile([128, 3 * NTAB], bf16, tag="c012")
            nc.scalar.copy(
                c012[:].rearrange("p (j c) -> p j c", j=3),
                G[:].rearrange("p (j c) -> p j c", j=3, c=1024)[:, :, 0:NTAB])
            # R3a singleton operand: S_a = x_a = G0 col a (singleton sets are
            # (a,a,a)), materialized rep-each-62 by ACT so the grid max can
            # run bf16 2x on DVE
            r3aS = wp.tile([128, RMAT * NPAIR_C], bf16, tag="r3aS")
            if RMAT > 0:
                nc.scalar.copy(
                    r3aS[:].rearrange("p (a q) -> p a q", a=RMAT),
                    G[:, 0:RMAT].unsqueeze(2)
                    .broadcast_to([128, RMAT, NPAIR_C]))
            c0 = c012[:, 0:NTAB]
            c1 = c012[:, NTAB:2 * NTAB]
            c2 = c012[:, 2 * NTAB:3 * NTAB]

            xcall = wp.tile([128, LPACK], bf16, tag="xcall")
            sm = xcall[:, 0:N_R1]
            mx = xcall[:, N_R1:N_R1 + N_R2]

            # R1 = min3, R2 = max3 (bf16 2x chain); pair mins live in t1
            t1 = wp.tile([128, N_R1], bf16, tag="t1")
            nc.vector.tensor_tensor(t1[:], c0[:, 0:N_R1], c1[:, 0:N_R1],
                                    Alu.min)
            # X = min3 over the dup-pair tail [714:838] (R3b grid operands)
            xs1 = wp.tile([128, NM_C * XPAD], bf16, tag="xs1")
            xs = wp.tile([128, NM_C * XPAD], bf16, tag="xs")
            nc.vector.tensor_tensor(xs1[:], c0[:, N_R1:NTAB],
                                    c1[:, N_R1:NTAB], Alu.min)
            nc.vector.tensor_tensor(xs[:], xs1[:], c2[:, N_R1:NTAB], Alu.min)
            nc.vector.tensor_tensor(sm, t1[:], c2[:, 0:N_R1], Alu.min)
            t2 = wp.tile([128, N_R2], bf16, tag="t2")
            nc.vector.tensor_tensor(t2[:], c0[:, 32:N_R1], c1[:, 32:N_R1],
                                    Alu.max)
            nc.vector.tensor_tensor(mx, t2[:], c2[:, 32:N_R1], Alu.max)

            # R3a grid [32 x 62]: max(S_a, P_p); pair operand = t1[32:94]
            # (pair sets are (i,j,j) so min3 == min(c0,c1))
            r3a = xcall[:, SEG["R3a"][0]:SEG["R3a"][1]].rearrange(
                "p (a q) -> p a q", a=32)
            pair_b = (t1[:, 32:32 + NPAIR_C].unsqueeze(1)
                      .broadcast_to([128, 32, NPAIR_C]))
            if RMAT > 0:
                nc.vector.tensor_tensor(
                    r3a[:, 0:RMAT],
                    r3aS[:].rearrange("p (a q) -> p a q", a=RMAT),
                    pair_b[:, 0:RMAT], Alu.max)
            if RMAT < 32:
                nc.vector.tensor_tensor(
                    r3a[:, RMAT:32],
                    sm[:, RMAT:32].unsqueeze(2)
                    .broadcast_to([128, 32 - RMAT, NPAIR_C]),
                    pair_b[:, RMAT:32], Alu.max)

            junk = jp.tile([128, LPACK], bf16, tag="junk")
            if not WSUM_STT:
                junk2 = jp.tile([128, LPACK], bf16, tag="junk2")
            # wsum chunk A: R1+R2+R3a, accumulates while R3b grids run
            if WSUM_STT:
                nc.vector.scalar_tensor_tensor(
                    junk[:, 0:WSPLIT], xcall[:, 0:WSPLIT], 1.0,
                    E[:, 0:WSPLIT], op0=Alu.mult, op1=Alu.mult,
                    accum_out=outd_b[:, t:t + 1])
            else:
                nc.vector.tensor_tensor(junk[:, 0:WSPLIT], xcall[:, 0:WSPLIT],
                                        E[:, 0:WSPLIT], Alu.mult)
                nc.scalar.activation(junk2[:, 0:WSPLIT], junk[:, 0:WSPLIT],
                                     Act.Copy, accum_out=outd_b[:, t:t + 1])

            # R3b 2-level triangle-split grids over X; the rep-each (u)
            # operand is materialized by ACT so the DVE max runs 2x
            xv = xs[:].rearrange("p (m t) -> p m t", m=NM_C)
            if R3B_MAT:
                r3bS = wp.tile([128, SEG["tB2"][1] - SEG["rAB"][0]], bf16,
                               tag="r3bS")
            off0 = SEG["rAB"][0]
            for name, ud, vd, uo, vo, _ in R3B_GRIDS:
                s, e = SEG[name]
                dst = xcall[:, s:e].rearrange("p (m u v) -> p m u v",
                                              m=NM_C, u=ud)
                u_b = (xv[:, :, uo:uo + ud].unsqueeze(3)
                       .broadcast_to([128, NM_C, ud, vd]))
                if R3B_MAT:
                    um = r3bS[:, s - off0:e - off0].rearrange(
                        "p (m u v) -> p m u v", m=NM_C, u=ud)
                    nc.scalar.copy(um, u_b)
                    u_b = um
                nc.vector.tensor_tensor(
                    dst, u_b,
                    xv[:, :, vo:vo + vd].unsqueeze(2)
                    .broadcast_to([128, NM_C, ud, vd]),
                    Alu.max)

            # wsum chunk B: R3b
            if WSUM_STT:
                nc.vector.scalar_tensor_tensor(
                    junk[:, WSPLIT:LPACK], xcall[:, WSPLIT:LPACK], 1.0,
                    E[:, WSPLIT:LPACK], op0=Alu.mult, op1=Alu.mult,
                    accum_out=outa_b[:, t:t + 1])
            else:
                nc.vector.tensor_tensor(junk[:, WSPLIT:LPACK],
                                        xcall[:, WSPLIT:LPACK],
                                        E[:, WSPLIT:LPACK], Alu.mult)
                nc.scalar.activation(junk2[:, WSPLIT:LPACK],
                                     junk[:, WSPLIT:LPACK],
                                     Act.Copy, accum_out=outa_b[:, t:t + 1])
            nc.vector.memset(outp_b[:, t:t + 1], 0.0)


def make_in_maps(x, pw):
    import ml_dtypes

    bf = ml_dtypes.bfloat16
    xT = np.ascontiguousarray(np.asarray(x, np.float32).T.astype(bf))
    in_maps = []
    for core in range(NCORES):
        oh = _onehots(core)  # [3, 32, NTAB]
        in_maps.append({
            "xT": xT,
            "oh": np.ascontiguousarray(
                oh.transpose(1, 0, 2).reshape(DIM, 3 * NTAB).astype(bf)),
            "pwe": _packed_pwe(core, pw).reshape(1, LPACK).astype(bf),
        })
    return in_maps


def kernel(x, preweight, idx):
    from concourse.bass_utils import run_bass_kernel_spmd

    x = np.ascontiguousarray(np.asarray(x, dtype=np.float32))
    pw = np.asarray(preweight, dtype=np.float32).reshape(-1)
    idx = np.asarray(idx)
    if not np.array_equal(idx, _expected_idx()):
        raise ValueError("idx does not match the expected antichain table")

    if "nc" not in _NC_CACHE:
        _NC_CACHE["nc"] = _build_nc()
    nc = _NC_CACHE["nc"]

    in_maps = make_in_maps(x, pw)
    res = run_bass_kernel_spmd(nc, in_maps, core_ids=list(range(NCORES)))
    total = np.zeros((B, 1), dtype=np.float64)
    for r in res.results:
        total += r["outd"].astype(np.float64)
        total += r["outa"].astype(np.float64)
        total += r["outp"].astype(np.float64)
    z = float(np.sum(np.exp(pw.astype(np.float64))))
    return (total / z).astype(np.float32)


if __name__ == "__main__":
    rng = np.random.default_rng(11)
    x = rng.standard_normal((B, DIM)).astype(np.float32)
    pw = rng.standard_normal((1, 40704)).astype(np.float32)
    out = kernel(x, pw, _expected_idx())
    print("out", out.shape, out[:4, 0])


# revision 23
# speedup vs baseline: 1.1880x; 1.1880x over previous
"""Trainium2 Bass kernel for nn_CI3addFrom01 (segment_reduce).

Reference computation:
    out[b] = sum_m softmax(preweight)[m] * max_k min_j x[b, idx[m,k,j]]
with M = 40704 antichains over DIM = 32.

Device formulation (M-axis sharded 8 ways; per core, per 128-row batch
tile, everything is one-hot matmuls + rectangular broadcast min/max +
weighted accumulation against a host-precomputed exp(preweight) table):
    G_j = xT.T @ OH_j (j=0,1,2) over 838 "set" columns
    SM = min3(G)[:714]; MX = max3(G)[32:714]; X = min3(G)[714:838]
    xcall regions, column-aligned with a host-packed weight row E:
      R1 [0:714]     = SM              1-group antichains
      R2 [714:1396]  = MX              ((i,),(j,)) / ((i,),(j,),(k,))
      R3a [1396:3380] max(S_a, P_p)    [32 x 62] singleton-vs-pair grid
      R3b [3380:5784] max(X_u, X_v)    2-level triangle-split pair-vs-pair
    weighted sum: acc[b] += sum_c E_c * xcall[b, c], E = exp(pw_packed)
    (host computes exp and the softmax denominator; invalid grid slots
    get E = 0, double-counted triangle slots get E/2)
DVE does the bf16-2x min/max chain, the grids, and the fused
multiply-accumulate (scalar_tensor_tensor, which real TRN2 runs in 2x_1p
mode); ACT does the fused PSUM->SBUF copy and materializes the R3a
singleton operand (rep-each-62) so that grid runs 2x on DVE; E arrives
pre-exponentiated via a broadcast DMA on the SP queue.  All R3b operand
slices sit at even bf16 offsets (32-slot padded X layout) to satisfy the
4B-alignment condition of the DVE 2x mode.
Host sums the per-core partials and divides by Z = sum(exp(pw)).
"""

import itertools
import math

import numpy as np

DIM = 32
B = 512
NCORES = 8
NPAIR_C = 62
NTRIP_C = 620
NM_C = 4
# per-m X slice padded to 32 slots (31 real pair-mins + 1 dummy whose
# packed weight is 0) so every R3b grid operand slice starts at an even
# bf16 column (4B-aligned — required for the DVE 2x_1p perf mode)
XPAD = 32
NTAB = 32 + NPAIR_C + NTRIP_C + NM_C * XPAD  # 842
NEG = -1e30

PAIRS = list(itertools.combinations(range(DIM), 2))
TRIPS = list(itertools.combinations(range(DIM), 3))
TRIPIDX = {t: i for i, t in enumerate(TRIPS)}

# R3b 2-level triangle split grids: (name, udim, vdim, uoff, voff, halved)
# offsets index into the per-m 32-slot padded X slice; all offsets/dims
# even.  A = slots [0:16], B = [16:32] (slot 31 = dummy)
R3B_GRIDS = [
    ("rAB", 16, 16, 0, 16, False),
    ("rA", 8, 8, 0, 8, False),
    ("tA1", 8, 8, 0, 0, True),
    ("tA2", 8, 8, 8, 8, True),
    ("rB", 8, 8, 16, 24, False),
    ("tB1", 8, 8, 16, 16, True),
    ("tB2", 8, 8, 24, 24, True),
]

SEG = {}
_off = 0
for _name, _ln in (
    [("R1a", 32), ("R1b", NPAIR_C), ("R1c", NTRIP_C),
     ("R2b", NPAIR_C), ("R2c", NTRIP_C),
     ("R3a", 32 * NPAIR_C)]
    + [(n, NM_C * u * v) for n, u, v, _, _, _ in R3B_GRIDS]
):
    SEG[_name] = (_off, _off + _ln)
    _off += _ln
LPACK = _off  # 5784
N_R1 = 32 + NPAIR_C + NTRIP_C  # 714
N_R2 = NPAIR_C + NTRIP_C       # 682

# --- engine split tuning knobs ---
# RMAT: R3a rows [0:RMAT] get their singleton operand materialized by ACT
# (rep-each-62 copy from PSUM) so the DVE grid max runs in bf16 2x mode;
# rows [RMAT:32] use the stride-0 broadcast operand at 1x.
RMAT = 32
# materialize the R3b grids' rep-each operand on ACT (unlocks DVE 2x for
# the R3b grid maxes) — pays now that WSUM_STT freed the ACT accumulate
# passes; verified faster in every reps-65 A/B round
R3B_MAT = True
# weighted sum via DVE scalar_tensor_tensor (real HW gives it 2x_1p, so it
# fuses the multiply and the reduce at the TT-mult price and frees ACT);
# False = DVE TT-mult + ACT activation-accumulate
WSUM_STT = True
# wsum chunk boundary: [0:WSPLIT] accumulated right after R3a,
# [WSPLIT:LPACK] after the R3b grids (both DVE TT-mult + ACT accum-copy).
WSPLIT = 32 + NPAIR_C + NTRIP_C + NPAIR_C + NTRIP_C + 32 * NPAIR_C  # 3380


def _others(m):
    return [x for x in range(DIM) if x != m]


def _table_sets(core):
    sets = [(i, i, i) for i in range(DIM)]
    for p in range(NPAIR_C * core, NPAIR_C * (core + 1)):
        i, j = PAIRS[p]
        sets.append((i, j, j))
    for q in range(NTRIP_C * core, NTRIP_C * (core + 1)):
        sets.append(TRIPS[q])
    for m in range(NM_C * core, NM_C * (core + 1)):
        for x in _others(m):
            sets.append((min(m, x), max(m, x), max(m, x)))
        sets.append((0, 0, 0))  # pad slot 31: any finite value, weight 0
    return sets


_HOST_CACHE = {}


def _onehots(core):
    if ("oh", core) in _HOST_CACHE:
        return _HOST_CACHE[("oh", core)]
    sets = np.asarray(_table_sets(core), dtype=np.int64)
    oh = np.zeros((3, DIM, NTAB), dtype=np.float32)
    for j in range(3):
        oh[j, sets[:, j], np.arange(NTAB)] = 1.0
    _HOST_CACHE[("oh", core)] = oh
    return oh


def _widx_r3a(core):
    if ("r3a", core) in _HOST_CACHE:
        return _HOST_CACHE[("r3a", core)]
    g = np.full((32, NPAIR_C), -1, dtype=np.int64)
    for a in range(32):
        for pl, p in enumerate(range(NPAIR_C * core, NPAIR_C * (core + 1))):
            b, c = PAIRS[p]
            if a == b or a == c:
                continue
            tri = tuple(sorted((a, b, c)))
            g[a, pl] = 1024 + 8 * TRIPIDX[tri] + 2 + tri.index(a)
    _HOST_CACHE[("r3a", core)] = g
    return g


def _r3b_widx_id(m, ou, ov):
    tri = tuple(sorted((m, ou, ov)))
    return 1024 + 8 * TRIPIDX[tri] + (7, 5, 6)[tri.index(m)]


def _widx_r3b(core):
    """Per 2-level grid: index table [NM_C, u, v] (-1 = invalid slot).
    Slot indices address the padded 32-slot others list; slot 31 (pad)
    and diagonal cells are invalid (weight 0)."""
    if ("r3b", core) in _HOST_CACHE:
        return _HOST_CACHE[("r3b", core)]
    grids = {}
    for name, ud, vd, uo, vo, halved in R3B_GRIDS:
        g = np.full((NM_C, ud, vd), -1, dtype=np.int64)
        for ml, m in enumerate(range(NM_C * core, NM_C * (core + 1))):
            ot = _others(m)
            for u in range(ud):
                for v in range(vd):
                    iu, iv = uo + u, vo + v
                    if iu >= 31 or iv >= 31:
                        continue
                    gu, gv = ot[iu], ot[iv]
                    if gu != gv:
                        g[ml, u, v] = _r3b_widx_id(m, gu, gv)
        grids[name] = (g, halved)
    _HOST_CACHE[("r3b", core)] = grids
    return grids


def _packed_pwe(core, pw):
    """exp of the packed preweight row: the weighted-sum table E."""
    pw = np.asarray(pw, dtype=np.float64).reshape(-1)
    out = np.full(LPACK, NEG, dtype=np.float64)
    if core == 0:
        out[SEG["R1a"][0]:SEG["R1a"][1]] = pw[0:32]
    p0, p1 = 32 + 2 * NPAIR_C * core, 32 + 2 * NPAIR_C * (core + 1)
    out[SEG["R1b"][0]:SEG["R1b"][1]] = pw[p0:p1:2]
    out[SEG["R2b"][0]:SEG["R2b"][1]] = pw[p0 + 1:p1 + 1:2]
    t0, t1 = 1024 + 8 * NTRIP_C * core, 1024 + 8 * NTRIP_C * (core + 1)
    out[SEG["R1c"][0]:SEG["R1c"][1]] = pw[t0:t1:8]
    out[SEG["R2c"][0]:SEG["R2c"][1]] = pw[t0 + 1:t1 + 1:8]
    items = [("R3a", _widx_r3a(core), False)]
    r3b = _widx_r3b(core)
    for name, _, _, _, _, _ in R3B_GRIDS:
        g, halved = r3b[name]
        items.append((name, g, halved))
    for name, grid, halved in items:
        flat = grid.reshape(-1)
        vals = np.full(flat.shape, NEG, dtype=np.float64)
        ok = flat >= 0
        vals[ok] = pw[flat[ok]]
        if halved:
            vals[ok] -= math.log(2.0)
        s, e = SEG[name]
        out[s:e] = vals
    return np.exp(out)


def _expected_idx():
    acs = [((i,),) for i in range(DIM)]
    for i, j in PAIRS:
        acs.append(((i, j),))
        acs.append(((i,), (j,)))
    for i, j, k in TRIPS:
        acs += [((i, j, k),), ((i,), (j,), (k,)), ((i,), (j, k)), ((j,), (i, k)),
                ((k,), (i, j)), ((i, j), (j, k)), ((i, k), (j, k)), ((i, j), (i, k))]
    idx = np.zeros((len(acs), 3, 3), dtype=np.int32)
    for m, ac in enumerate(acs):
        groups = [list(g) + [g[-1]] * (3 - len(g)) for g in ac]
        while len(groups) < 3:
            groups.append(groups[-1])
        idx[m] = np.array(groups, dtype=np.int32)
    return idx


_NC_CACHE = {}

_WP_BUFS = 3
_JP_BUFS = 2


def _build_nc(reps=1):
    import concourse.mybir as mybir
    from concourse import bacc
    from concourse.tile import TileContext

    f32 = mybir.dt.float32
    bf16 = mybir.dt.bfloat16

    nc = bacc.Bacc(None, target_bir_lowering=False, debug=False)
    xT_d = nc.dram_tensor("xT", [DIM, B], bf16, kind="ExternalInput")
    oh_d = nc.dram_tensor("oh", [DIM, 3 * NTAB], bf16, kind="ExternalInput")
    pwe_d = nc.dram_tensor("pwe", [1, LPACK], bf16, kind="ExternalInput")
    outd_d = nc.dram_tensor("outd", [B, 1], f32, kind="ExternalOutput")
    outa_d = nc.dram_tensor("outa", [B, 1], f32, kind="ExternalOutput")
    outp_d = nc.dram_tensor("outp", [B, 1], f32, kind="ExternalOutput")

    with TileContext(nc) as tc:
        with (
            tc.tile_pool(name="const", bufs=1) as cp,
            tc.tile_pool(name="ep", bufs=2) as ep,
            tc.tile_pool(name="work", bufs=_WP_BUFS) as wp,
            tc.tile_pool(name="junkp", bufs=_JP_BUFS) as jp,
            tc.tile_pool(name="pg", bufs=1, space="PSUM") as pg_pool,
        ):
            oh_t = cp.tile([DIM, 3 * NTAB], bf16)
            xt_t = cp.tile([DIM, B], bf16)
            outd_b = cp.tile([128, 4], f32)
            outa_b = cp.tile([128, 4], f32)
            outp_b = cp.tile([128, 4], f32)
            nc.sync.dma_start(oh_t[:], oh_d[:])
            nc.sync.dma_start(xt_t[:], xT_d[:])

            rep_blocks(nc, tc, mybir, f32, bf16, cp, ep, wp, jp, pg_pool,
                       oh_t, xt_t, pwe_d, outd_b, outa_b, outp_b, reps)

            for t in range(4):
                nc.sync.dma_start(outd_d[t * 128:(t + 1) * 128, :],
                                  outd_b[:, t:t + 1])
                nc.sync.dma_start(outa_d[t * 128:(t + 1) * 128, :],
                                  outa_b[:, t:t + 1])
                nc.sync.dma_start(outp_d[t * 128:(t + 1) * 128, :],
                                  outp_b[:, t:t + 1])
    nc.finalize()
    return nc


def rep_blocks(nc, tc, mybir, f32, bf16, cp, ep, wp, jp, pg_pool,
               oh_t, xt_t, pwe_d, outd_b, outa_b, outp_b, reps):
    Alu = mybir.AluOpType
    Act = mybir.ActivationFunctionType
    for _rep in range(reps):
        # E = exp(pw_packed) broadcast to 128 partitions via DMA on the SP
        # queue (keep the ACT queue free); double-buffered across reps
        E = ep.tile([128, LPACK], bf16, tag="E")
        nc.sync.dma_start(E[:], pwe_d[0:1, :].broadcast_to([128, LPACK]))

        for t in range(4):
            # G: one PSUM tile, j-blocks at col j*1024 so every 512-col
            # matmul chunk is PSUM-bank aligned
            G = pg_pool.tile([128, 3 * 1024], f32, tag="G")
            for j in range(3):
                for s in range(0, NTAB, 512):
                    e = min(s + 512, NTAB)
                    nc.tensor.matmul(
                        G[:, j * 1024 + s: j * 1024 + e],
                        xt_t[:, t * 128:(t + 1) * 128],
                        oh_t[:, j * NTAB + s: j * NTAB + e],
                        start=True, stop=True)

            # single fused PSUM->SBUF bf16 copy of all three j-blocks
            c012 = wp.tile([128, 3 * NTAB], bf16, tag="c012")
            nc.scalar.copy(
                c012[:].rearrange("p (j c) -> p j c", j=3),
                G[:].rearrange("p (j c) -> p j c", j=3, c=1024)[:, :, 0:NTAB])
            # R3a singleton operand: S_a = x_a = G0 col a (singleton sets are
            # (a,a,a)), materialized rep-each-62 by ACT so the grid max can
            # run bf16 2x on DVE
            r3aS = wp.tile([128, RMAT * NPAIR_C], bf16, tag="r3aS")
            if RMAT > 0:
                nc.scalar.copy(
                    r3aS[:].rearrange("p (a q) -> p a q", a=RMAT),
                    G[:, 0:RMAT].unsqueeze(2)
                    .broadcast_to([128, RMAT, NPAIR_C]))
            c0 = c012[:, 0:NTAB]
            c1 = c012[:, NTAB:2 * NTAB]
            c2 = c012[:, 2 * NTAB:3 * NTAB]

            xcall = wp.tile([128, LPACK], bf16, tag="xcall")
            sm = xcall[:, 0:N_R1]
            mx = xcall[:, N_R1:N_R1 + N_R2]

            # R1 = min3, R2 = max3 (bf16 2x chain); pair mins live in t1
            t1 = wp.tile([128, N_R1], bf16, tag="t1")
            nc.vector.tensor_tensor(t1[:], c0[:, 0:N_R1], c1[:, 0:N_R1],
                                    Alu.min)
            # X = min3 over the dup-pair tail [714:838] (R3b grid operands)
            xs1 = wp.tile([128, NM_C * XPAD], bf16, tag="xs1")
            xs = wp.tile([128, NM_C * XPAD], bf16, tag="xs")
            nc.vector.tensor_tensor(xs1[:], c0[:, N_R1:NTAB],
                                    c1[:, N_R1:NTAB], Alu.min)
            nc.vector.tensor_tensor(xs[:], xs1[:], c2[:, N_R1:NTAB], Alu.min)
            nc.vector.tensor_tensor(sm, t1[:], c2[:, 0:N_R1], Alu.min)
            t2 = wp.tile([128, N_R2], bf16, tag="t2")
            nc.vector.tensor_tensor(t2[:], c0[:, 32:N_R1], c1[:, 32:N_R1],
                                    Alu.max)
            nc.vector.tensor_tensor(mx, t2[:], c2[:, 32:N_R1], Alu.max)

            # R3a grid [32 x 62]: max(S_a, P_p); pair operand = t1[32:94]
            # (pair sets are (i,j,j) so min3 == min(c0,c1))
            r3a = xcall[:, SEG["R3a"][0]:SEG["R3a"][1]].rearrange(
                "p (a q) -> p a q", a=32)
            pair_b = (t1[:, 32:32 + NPAIR_C].unsqueeze(1)
                      .broadcast_to([128, 32, NPAIR_C]))
            if RMAT > 0:
                nc.vector.tensor_tensor(
                    r3a[:, 0:RMAT],
                    r3aS[:].rearrange("p (a q) -> p a q", a=RMAT),
                    pair_b[:, 0:RMAT], Alu.max)
            if RMAT < 32:
                nc.vector.tensor_tensor(
                    r3a[:, RMAT:32],
                    sm[:, RMAT:32].unsqueeze(2)
                    .broadcast_to([128, 32 - RMAT, NPAIR_C]),
                    pair_b[:, RMAT:32], Alu.max)

            junk = jp.tile([128, LPACK], bf16, tag="junk")
            if not WSUM_STT:
                junk2 = jp.tile([128, LPACK], bf16, tag="junk2")
            # wsum chunk A: R1+R2+R3a, accumulates while R3b grids run
            if WSUM_STT:
                nc.vector.scalar_tensor_tensor(
                    junk[:, 0:WSPLIT], xcall[:, 0:WSPLIT], 1.0,
                    E[:, 0:WSPLIT], op0=Alu.mult, op1=Alu.mult,
                    accum_out=outd_b[:, t:t + 1])
            else:
                nc.vector.tensor_tensor(junk[:, 0:WSPLIT], xcall[:, 0:WSPLIT],
                                        E[:, 0:WSPLIT], Alu.mult)
                nc.scalar.activation(junk2[:, 0:WSPLIT], junk[:, 0:WSPLIT],
                                     Act.Copy, accum_out=outd_b[:, t:t + 1])

            # R3b 2-level triangle-split grids over X; the rep-each (u)
            # operand is materialized by ACT so the DVE max runs 2x
            xv = xs[:].rearrange("p (m t) -> p m t", m=NM_C)
            if R3B_MAT:
                r3bS = wp.tile([128, SEG["tB2"][1] - SEG["rAB"][0]], bf16,
                               tag="r3bS")
            off0 = SEG["rAB"][0]
            for name, ud, vd, uo, vo, _ in R3B_GRIDS:
                s, e = SEG[name]
                dst = xcall[:, s:e].rearrange("p (m u v) -> p m u v",
                                              m=NM_C, u=ud)
                u_b = (xv[:, :, uo:uo + ud].unsqueeze(3)
                       .broadcast_to([128, NM_C, ud, vd]))
                if R3B_MAT:
                    um = r3bS[:, s - off0:e - off0].rearrange(
                        "p (m u v) -> p m u v", m=NM_C, u=ud)
                    nc.scalar.copy(um, u_b)
                    u_b = um
                nc.vector.tensor_tensor(
                    dst, u_b,
                    xv[:, :, vo:vo + vd].unsqueeze(2)
                    .broadcast_to([128, NM_C, ud, vd]),
                    Alu.max)

            # wsum chunk B: R3b
            if WSUM_STT:
                nc.vector.scalar_tensor_tensor(
                    junk[:, WSPLIT:LPACK], xcall[:, WSPLIT:LPACK], 1.0,
                    E[:, WSPLIT:LPACK], op0=Alu.mult, op1=Alu.mult,
                    accum_out=outa_b[:, t:t + 1])
            else:
                nc.vector.tensor_tensor(junk[:, WSPLIT:LPACK],
                                        xcall[:, WSPLIT:LPACK],
                                        E[:, WSPLIT:LPACK], Alu.mult)
                nc.scalar.activation(junk2[:, WSPLIT:LPACK],
                                     junk[:, WSPLIT:LPACK],
                                     Act.Copy, accum_out=outa_b[:, t:t + 1])
            nc.vector.memset(outp_b[:, t:t + 1], 0.0)


def make_in_maps(x, pw):
    import ml_dtypes

    bf = ml_dtypes.bfloat16
    xT = np.ascontiguousarray(np.asarray(x, np.float32).T.astype(bf))
    in_maps = []
    for core in range(NCORES):
        oh = _onehots(core)  # [3, 32, NTAB]
        in_maps.append({
            "xT": xT,
            "oh": np.ascontiguousarray(
                oh.transpose(1, 0, 2).reshape(DIM, 3 * NTAB).astype(bf)),
            "pwe": _packed_pwe(core, pw).reshape(1, LPACK).astype(bf),
        })
    return in_maps


def kernel(x, preweight, idx):
    from concourse.bass_utils import run_bass_kernel_spmd

    x = np.ascontiguousarray(np.asarray(x, dtype=np.float32))
    pw = np.asarray(preweight, dtype=np.float32).reshape(-1)
    idx = np.asarray(idx)
    if not np.array_equal(idx, _expected_idx()):
        raise ValueError("idx does not match the expected antichain table")

    if "nc" not in _NC_CACHE:
        _NC_CACHE["nc"] = _build_nc()
    nc = _NC_CACHE["nc"]

    in_maps = make_in_maps(x, pw)
    res = run_bass_kernel_spmd(nc, in_maps, core_ids=list(range(NCORES)))
    total = np.zeros((B, 1), dtype=np.float64)
    for r in res.results:
        total += r["outd"].astype(np.float64)
        total += r["outa"].astype(np.float64)
        total += r["outp"].astype(np.float64)
    z = float(np.sum(np.exp(pw.astype(np.float64))))
    return (total / z).astype(np.float32)


if __name__ == "__main__":
    rng = np.random.default_rng(11)
    x = rng.standard_normal((B, DIM)).astype(np.float32)
    pw = rng.standard_normal((1, 40704)).astype(np.float32)
    out = kernel(x, pw, _expected_idx())
    print("out", out.shape, out[:4, 0])


# revision 33
# speedup vs baseline: 1.2547x; 1.0561x over previous
"""Trainium2 Bass kernel for nn_CI3addFrom01 (segment_reduce).

Reference computation:
    out[b] = sum_m softmax(preweight)[m] * max_k min_j x[b, idx[m,k,j]]
with M = 40704 antichains over DIM = 32.

Device formulation (M-axis sharded 8 ways; per core, per 128-row batch
tile, everything is one-hot matmuls + rectangular broadcast min/max +
weighted accumulation against a host-precomputed exp(preweight) table):
    G_j = xT.T @ OH_j (j=0,1,2) over 842 "set" columns
    SM = min3(G)[:714]; MX = max3(G)[32:714]; X = min3(G)[714:842]
    xcall regions, column-aligned with a host-packed weight row E:
      R1 [0:714]     = SM              1-group antichains
      R2 [714:1396]  = MX              ((i,),(j,)) / ((i,),(j,),(k,))
      R3a [1396:3380] max(S_a, P_p)    [32 x 62] singleton-vs-pair grid
      R3b [3380:5940] max(X_u, X_v)    2-level triangle-split pair-vs-pair
    weighted sum: acc[b] += sum_c E_c * xcall[b, c], E = exp(pw_packed)
    (host computes exp and the softmax denominator; invalid grid slots
    get E = 0, double-counted triangle slots get E/2)
DVE does the bf16-2x min/max chain, the grids, and the fused
multiply-accumulate (scalar_tensor_tensor, which real TRN2 runs in 2x_1p
mode); ACT does the fused PSUM->SBUF copy and materializes the rep-each
grid operands (R3a singleton, R3b u-operand) so every grid max runs 2x
on DVE; E arrives pre-exponentiated via a broadcast DMA on the SP queue.  All R3b operand
slices sit at even bf16 offsets (32-slot padded X layout) to satisfy the
4B-alignment condition of the DVE 2x mode.
Host sums the per-core partials and divides by Z = sum(exp(pw)).
"""

import itertools
import math

import numpy as np

DIM = 32
B = 512
NCORES = 8
NPAIR_C = 62
NTRIP_C = 620
NM_C = 4
# per-m X slice padded to 32 slots (31 real pair-mins + 1 dummy whose
# packed weight is 0) so every R3b grid operand slice starts at an even
# bf16 column (4B-aligned — required for the DVE 2x_1p perf mode)
XPAD = 32
NTAB = 32 + NPAIR_C + NTRIP_C + NM_C * XPAD  # 842
NEG = -1e30

PAIRS = list(itertools.combinations(range(DIM), 2))
TRIPS = list(itertools.combinations(range(DIM), 3))
TRIPIDX = {t: i for i, t in enumerate(TRIPS)}

# R3b 2-level triangle split grids: (name, udim, vdim, uoff, voff, halved)
# offsets index into the per-m 32-slot padded X slice; all offsets/dims
# even.  A = slots [0:16], B = [16:32] (slot 31 = dummy)
R3B_GRIDS = [
    ("rAB", 16, 16, 0, 16, False),
    ("rA", 8, 8, 0, 8, False),
    ("tA1", 8, 8, 0, 0, True),
    ("tA2", 8, 8, 8, 8, True),
    ("rB", 8, 8, 16, 24, False),
    ("tB1", 8, 8, 16, 16, True),
    ("tB2", 8, 8, 24, 24, True),
]

SEG = {}
_off = 0
for _name, _ln in (
    [("R1a", 32), ("R1b", NPAIR_C), ("R1c", NTRIP_C),
     ("R2b", NPAIR_C), ("R2c", NTRIP_C),
     ("R3a", 32 * NPAIR_C)]
    + [(n, NM_C * u * v) for n, u, v, _, _, _ in R3B_GRIDS]
):
    SEG[_name] = (_off, _off + _ln)
    _off += _ln
LPACK = _off  # 5940
N_R1 = 32 + NPAIR_C + NTRIP_C  # 714
N_R2 = NPAIR_C + NTRIP_C       # 682

# --- engine split tuning knobs ---
# RMAT: R3a rows [0:RMAT] get their singleton operand materialized by ACT
# (rep-each-62 copy from PSUM) so the DVE grid max runs in bf16 2x mode;
# rows [RMAT:32] use the stride-0 broadcast operand at 1x.
RMAT = 32
# materialize the R3b grids' rep-each operand on ACT (unlocks DVE 2x for
# the R3b grid maxes) — pays now that WSUM_STT freed the ACT accumulate
# passes; verified faster in every reps-65 A/B round
R3B_MAT = True
# weighted sum via DVE scalar_tensor_tensor (real HW gives it 2x_1p, so it
# fuses the multiply and the reduce at the TT-mult price and frees ACT);
# False = DVE TT-mult + ACT activation-accumulate
WSUM_STT = True
# wsum chunk boundary: [0:WSPLIT] accumulated right after R3a,
# [WSPLIT:LPACK] after the R3b grids (both DVE TT-mult + ACT accum-copy).
WSPLIT = 32 + NPAIR_C + NTRIP_C + NPAIR_C + NTRIP_C + 32 * NPAIR_C  # 3380


def _others(m):
    return [x for x in range(DIM) if x != m]


def _table_sets(core):
    sets = [(i, i, i) for i in range(DIM)]
    for p in range(NPAIR_C * core, NPAIR_C * (core + 1)):
        i, j = PAIRS[p]
        sets.append((i, j, j))
    for q in range(NTRIP_C * core, NTRIP_C * (core + 1)):
        sets.append(TRIPS[q])
    for m in range(NM_C * core, NM_C * (core + 1)):
        for x in _others(m):
            sets.append((min(m, x), max(m, x), max(m, x)))
        sets.append((0, 0, 0))  # pad slot 31: any finite value, weight 0
    return sets


_HOST_CACHE = {}


def _onehots(core):
    if ("oh", core) in _HOST_CACHE:
        return _HOST_CACHE[("oh", core)]
    sets = np.asarray(_table_sets(core), dtype=np.int64)
    oh = np.zeros((3, DIM, NTAB), dtype=np.float32)
    for j in range(3):
        oh[j, sets[:, j], np.arange(NTAB)] = 1.0
    _HOST_CACHE[("oh", core)] = oh
    return oh


def _widx_r3a(core):
    if ("r3a", core) in _HOST_CACHE:
        return _HOST_CACHE[("r3a", core)]
    g = np.full((32, NPAIR_C), -1, dtype=np.int64)
    for a in range(32):
        for pl, p in enumerate(range(NPAIR_C * core, NPAIR_C * (core + 1))):
            b, c = PAIRS[p]
            if a == b or a == c:
                continue
            tri = tuple(sorted((a, b, c)))
            g[a, pl] = 1024 + 8 * TRIPIDX[tri] + 2 + tri.index(a)
    _HOST_CACHE[("r3a", core)] = g
    return g


def _r3b_widx_id(m, ou, ov):
    tri = tuple(sorted((m, ou, ov)))
    return 1024 + 8 * TRIPIDX[tri] + (7, 5, 6)[tri.index(m)]


def _widx_r3b(core):
    """Per 2-level grid: index table [NM_C, u, v] (-1 = invalid slot).
    Slot indices address the padded 32-slot others list; slot 31 (pad)
    and diagonal cells are invalid (weight 0)."""
    if ("r3b", core) in _HOST_CACHE:
        return _HOST_CACHE[("r3b", core)]
    grids = {}
    for name, ud, vd, uo, vo, halved in R3B_GRIDS:
        g = np.full((NM_C, ud, vd), -1, dtype=np.int64)
        for ml, m in enumerate(range(NM_C * core, NM_C * (core + 1))):
            ot = _others(m)
            for u in range(ud):
                for v in range(vd):
                    iu, iv = uo + u, vo + v
                    if iu >= 31 or iv >= 31:
                        continue
                    gu, gv = ot[iu], ot[iv]
                    if gu != gv:
                        g[ml, u, v] = _r3b_widx_id(m, gu, gv)
        grids[name] = (g, halved)
    _HOST_CACHE[("r3b", core)] = grids
    return grids


def _packed_pwe(core, pw):
    """exp of the packed preweight row: the weighted-sum table E."""
    pw = np.asarray(pw, dtype=np.float64).reshape(-1)
    out = np.full(LPACK, NEG, dtype=np.float64)
    if core == 0:
        out[SEG["R1a"][0]:SEG["R1a"][1]] = pw[0:32]
    p0, p1 = 32 + 2 * NPAIR_C * core, 32 + 2 * NPAIR_C * (core + 1)
    out[SEG["R1b"][0]:SEG["R1b"][1]] = pw[p0:p1:2]
    out[SEG["R2b"][0]:SEG["R2b"][1]] = pw[p0 + 1:p1 + 1:2]
    t0, t1 = 1024 + 8 * NTRIP_C * core, 1024 + 8 * NTRIP_C * (core + 1)
    out[SEG["R1c"][0]:SEG["R1c"][1]] = pw[t0:t1:8]
    out[SEG["R2c"][0]:SEG["R2c"][1]] = pw[t0 + 1:t1 + 1:8]
    items = [("R3a", _widx_r3a(core), False)]
    r3b = _widx_r3b(core)
    for name, _, _, _, _, _ in R3B_GRIDS:
        g, halved = r3b[name]
        items.append((name, g, halved))
    for name, grid, halved in items:
        flat = grid.reshape(-1)
        vals = np.full(flat.shape, NEG, dtype=np.float64)
        ok = flat >= 0
        vals[ok] = pw[flat[ok]]
        if halved:
            vals[ok] -= math.log(2.0)
        s, e = SEG[name]
        out[s:e] = vals
    return np.exp(out)


def _expected_idx():
    acs = [((i,),) for i in range(DIM)]
    for i, j in PAIRS:
        acs.append(((i, j),))
        acs.append(((i,), (j,)))
    for i, j, k in TRIPS:
        acs += [((i, j, k),), ((i,), (j,), (k,)), ((i,), (j, k)), ((j,), (i, k)),
                ((k,), (i, j)), ((i, j), (j, k)), ((i, k), (j, k)), ((i, j), (i, k))]
    idx = np.zeros((len(acs), 3, 3), dtype=np.int32)
    for m, ac in enumerate(acs):
        groups = [list(g) + [g[-1]] * (3 - len(g)) for g in ac]
        while len(groups) < 3:
            groups.append(groups[-1])
        idx[m] = np.array(groups, dtype=np.int32)
    return idx


_NC_CACHE = {}

_WP_BUFS = 3
_JP_BUFS = 2


def _build_nc(reps=1):
    import concourse.mybir as mybir
    from concourse import bacc
    from concourse.tile import TileContext

    f32 = mybir.dt.float32
    bf16 = mybir.dt.bfloat16

    nc = bacc.Bacc(None, target_bir_lowering=False, debug=False)
    xT_d = nc.dram_tensor("xT", [DIM, B], bf16, kind="ExternalInput")
    oh_d = nc.dram_tensor("oh", [DIM, 3 * NTAB], bf16, kind="ExternalInput")
    pwe_d = nc.dram_tensor("pwe", [1, LPACK], bf16, kind="ExternalInput")
    outd_d = nc.dram_tensor("outd", [B, 1], f32, kind="ExternalOutput")
    outa_d = nc.dram_tensor("outa", [B, 1], f32, kind="ExternalOutput")
    outp_d = nc.dram_tensor("outp", [B, 1], f32, kind="ExternalOutput")

    with TileContext(nc) as tc:
        with (
            tc.tile_pool(name="const", bufs=1) as cp,
            tc.tile_pool(name="ep", bufs=2) as ep,
            tc.tile_pool(name="work", bufs=_WP_BUFS) as wp,
            tc.tile_pool(name="junkp", bufs=_JP_BUFS) as jp,
            tc.tile_pool(name="pg", bufs=1, space="PSUM") as pg_pool,
        ):
            oh_t = cp.tile([DIM, 3 * NTAB], bf16)
            xt_t = cp.tile([DIM, B], bf16)
            outd_b = cp.tile([128, 4], f32)
            outa_b = cp.tile([128, 4], f32)
            outp_b = cp.tile([128, 4], f32)
            nc.sync.dma_start(oh_t[:], oh_d[:])
            nc.sync.dma_start(xt_t[:], xT_d[:])

            rep_blocks(nc, tc, mybir, f32, bf16, cp, ep, wp, jp, pg_pool,
                       oh_t, xt_t, pwe_d, outd_b, outa_b, outp_b, reps)

            for t in range(4):
                nc.sync.dma_start(outd_d[t * 128:(t + 1) * 128, :],
                                  outd_b[:, t:t + 1])
                nc.sync.dma_start(outa_d[t * 128:(t + 1) * 128, :],
                                  outa_b[:, t:t + 1])
                nc.sync.dma_start(outp_d[t * 128:(t + 1) * 128, :],
                                  outp_b[:, t:t + 1])
    nc.finalize()
    return nc


def rep_blocks(nc, tc, mybir, f32, bf16, cp, ep, wp, jp, pg_pool,
               oh_t, xt_t, pwe_d, outd_b, outa_b, outp_b, reps):
    Alu = mybir.AluOpType
    Act = mybir.ActivationFunctionType
    for _rep in range(reps):
        # E = exp(pw_packed) broadcast to 128 partitions via DMA on the SP
        # queue (keep the ACT queue free); double-buffered across reps
        E = ep.tile([128, LPACK], bf16, tag="E")
        nc.sync.dma_start(E[:], pwe_d[0:1, :].broadcast_to([128, LPACK]))

        for t in range(4):
            # G: one PSUM tile, j-blocks at col j*1024 so every 512-col
            # matmul chunk is PSUM-bank aligned
            G = pg_pool.tile([128, 3 * 1024], f32, tag="G")
            for j in range(3):
                for s in range(0, NTAB, 512):
                    e = min(s + 512, NTAB)
                    nc.tensor.matmul(
                        G[:, j * 1024 + s: j * 1024 + e],
                        xt_t[:, t * 128:(t + 1) * 128],
                        oh_t[:, j * NTAB + s: j * NTAB + e],
                        start=True, stop=True)

            # single fused PSUM->SBUF bf16 copy of all three j-blocks
            c012 = wp.tile([128, 3 * NTAB], bf16, tag="c012")
            nc.scalar.copy(
                c012[:].rearrange("p (j c) -> p j c", j=3),
                G[:].rearrange("p (j c) -> p j c", j=3, c=1024)[:, :, 0:NTAB])
            # R3a singleton operand: S_a = x_a = G0 col a (singleton sets are
            # (a,a,a)), materialized rep-each-62 by ACT so the grid max can
            # run bf16 2x on DVE
            r3aS = wp.tile([128, RMAT * NPAIR_C], bf16, tag="r3aS")
            if RMAT > 0:
                nc.scalar.copy(
                    r3aS[:].rearrange("p (a q) -> p a q", a=RMAT),
                    G[:, 0:RMAT].unsqueeze(2)
                    .broadcast_to([128, RMAT, NPAIR_C]))
            c0 = c012[:, 0:NTAB]
            c1 = c012[:, NTAB:2 * NTAB]
            c2 = c012[:, 2 * NTAB:3 * NTAB]

            xcall = wp.tile([128, LPACK], bf16, tag="xcall")
            sm = xcall[:, 0:N_R1]
            mx = xcall[:, N_R1:N_R1 + N_R2]

            # R1 = min3, R2 = max3 (bf16 2x chain); pair mins live in t1
            t1 = wp.tile([128, N_R1], bf16, tag="t1")
            nc.vector.tensor_tensor(t1[:], c0[:, 0:N_R1], c1[:, 0:N_R1],
                                    Alu.min)
            # X = min3 over the dup-pair tail [714:838] (R3b grid operands)
            xs1 = wp.tile([128, NM_C * XPAD], bf16, tag="xs1")
            xs = wp.tile([128, NM_C * XPAD], bf16, tag="xs")
            nc.vector.tensor_tensor(xs1[:], c0[:, N_R1:NTAB],
                                    c1[:, N_R1:NTAB], Alu.min)
            nc.vector.tensor_tensor(xs[:], xs1[:], c2[:, N_R1:NTAB], Alu.min)
            nc.vector.tensor_tensor(sm, t1[:], c2[:, 0:N_R1], Alu.min)
            t2 = wp.tile([128, N_R2], bf16, tag="t2")
            nc.vector.tensor_tensor(t2[:], c0[:, 32:N_R1], c1[:, 32:N_R1],
                                    Alu.max)
            nc.vector.tensor_tensor(mx, t2[:], c2[:, 32:N_R1], Alu.max)

            # R3a grid [32 x 62]: max(S_a, P_p); pair operand = t1[32:94]
            # (pair sets are (i,j,j) so min3 == min(c0,c1))
            r3a = xcall[:, SEG["R3a"][0]:SEG["R3a"][1]].rearrange(
                "p (a q) -> p a q", a=32)
            pair_b = (t1[:, 32:32 + NPAIR_C].unsqueeze(1)
                      .broadcast_to([128, 32, NPAIR_C]))
            if RMAT > 0:
                nc.vector.tensor_tensor(
                    r3a[:, 0:RMAT],
                    r3aS[:].rearrange("p (a q) -> p a q", a=RMAT),
                    pair_b[:, 0:RMAT], Alu.max)
            if RMAT < 32:
                nc.vector.tensor_tensor(
                    r3a[:, RMAT:32],
                    sm[:, RMAT:32].unsqueeze(2)
                    .broadcast_to([128, 32 - RMAT, NPAIR_C]),
                    pair_b[:, RMAT:32], Alu.max)

            junk = jp.tile([128, LPACK], bf16, tag="junk")
            if not WSUM_STT:
                junk2 = jp.tile([128, LPACK], bf16, tag="junk2")
            # wsum chunk A: R1+R2+R3a, accumulates while R3b grids run
            if WSUM_STT:
                nc.vector.scalar_tensor_tensor(
                    junk[:, 0:WSPLIT], xcall[:, 0:WSPLIT], 1.0,
                    E[:, 0:WSPLIT], op0=Alu.mult, op1=Alu.mult,
                    accum_out=outd_b[:, t:t + 1])
            else:
                nc.vector.tensor_tensor(junk[:, 0:WSPLIT], xcall[:, 0:WSPLIT],
                                        E[:, 0:WSPLIT], Alu.mult)
                nc.scalar.activation(junk2[:, 0:WSPLIT], junk[:, 0:WSPLIT],
                                     Act.Copy, accum_out=outd_b[:, t:t + 1])

            # R3b 2-level triangle-split grids over X; the rep-each (u)
            # operand is materialized by ACT so the DVE max runs 2x
            xv = xs[:].rearrange("p (m t) -> p m t", m=NM_C)
            if R3B_MAT:
                r3bS = wp.tile([128, SEG["tB2"][1] - SEG["rAB"][0]], bf16,
                               tag="r3bS")
            off0 = SEG["rAB"][0]
            for name, ud, vd, uo, vo, _ in R3B_GRIDS:
                s, e = SEG[name]
                dst = xcall[:, s:e].rearrange("p (m u v) -> p m u v",
                                              m=NM_C, u=ud)
                u_b = (xv[:, :, uo:uo + ud].unsqueeze(3)
                       .broadcast_to([128, NM_C, ud, vd]))
                if R3B_MAT:
                    um = r3bS[:, s - off0:e - off0].rearrange(
                        "p (m u v) -> p m u v", m=NM_C, u=ud)
                    nc.scalar.copy(um, u_b)
                    u_b = um
                nc.vector.tensor_tensor(
                    dst, u_b,
                    xv[:, :, vo:vo + vd].unsqueeze(2)
                    .broadcast_to([128, NM_C, ud, vd]),
                    Alu.max)

            # wsum chunk B: R3b
            if WSUM_STT:
                nc.vector.scalar_tensor_tensor(
                    junk[:, WSPLIT:LPACK], xcall[:, WSPLIT:LPACK], 1.0,
                    E[:, WSPLIT:LPACK], op0=Alu.mult, op1=Alu.mult,
                    accum_out=outa_b[:, t:t + 1])
            else:
                nc.vector.tensor_tensor(junk[:, WSPLIT:LPACK],
                                        xcall[:, WSPLIT:LPACK],
                                        E[:, WSPLIT:LPACK], Alu.mult)
                nc.scalar.activation(junk2[:, WSPLIT:LPACK],
                                     junk[:, WSPLIT:LPACK],
                                     Act.Copy, accum_out=outa_b[:, t:t + 1])
            nc.vector.memset(outp_b[:, t:t + 1], 0.0)


def make_in_maps(x, pw):
    import ml_dtypes

    bf = ml_dtypes.bfloat16
    xT = np.ascontiguousarray(np.asarray(x, np.float32).T.astype(bf))
    in_maps = []
    for core in range(NCORES):
        oh = _onehots(core)  # [3, 32, NTAB]
        in_maps.append({
            "xT": xT,
            "oh": np.ascontiguousarray(
                oh.transpose(1, 0, 2).reshape(DIM, 3 * NTAB).astype(bf)),
            "pwe": _packed_pwe(core, pw).reshape(1, LPACK).astype(bf),
        })
    return in_maps


def kernel(x, preweight, idx):
    from concourse.bass_utils import run_bass_kernel_spmd

    x = np.ascontiguousarray(np.asarray(x, dtype=np.float32))
    pw = np.asarray(preweight, dtype=np.float32).reshape(-1)
    idx = np.asarray(idx)
    if not np.array_equal(idx, _expected_idx()):
        raise ValueError("idx does not match the expected antichain table")

    if "nc" not in _NC_CACHE:
        _NC_CACHE["nc"] = _build_nc()
    nc = _NC_CACHE["nc"]

    in_maps = make_in_maps(x, pw)
    res = run_bass_kernel_spmd(nc, in_maps, core_ids=list(range(NCORES)))
    total = np.zeros((B, 1), dtype=np.float64)
    for r in res.results:
        total += r["outd"].astype(np.float64)
        total += r["outa"].astype(np.float64)
        total += r["outp"].astype(np.float64)
    z = float(np.sum(np.exp(pw.astype(np.float64))))
    return (total / z).astype(np.float32)


if __name__ == "__main__":
    rng = np.random.default_rng(11)
    x = rng.standard_normal((B, DIM)).astype(np.float32)
    pw = rng.standard_normal((1, 40704)).astype(np.float32)
    out = kernel(x, pw, _expected_idx())
    print("out", out.shape, out[:4, 0])
